# revision 4
# baseline (speedup 1.0000x reference)
"""GatNet on 8 Trainium2 NeuronCores (Bass/Tile).

4-layer GAT (8 heads) + mean/max graph pooling + FC + log_softmax.

Distribution: edges partitioned by destination node across the 8 cores
(each core owns a contiguous range of 12544 destination nodes and all edges
into them). Per layer:
  - node phase (sharded): h = x @ W, als = x @ (W a_src), ad = x @ (W a_dst)
    for own nodes; h+als go into a 136-col row table that is AllGathered and
    expanded to 512B-strided rows (dma_gather elem_size must be a multiple of
    256B); ad goes into a compact local per-shard table (dst side is local).
  - edge phase: per 128-dst-node window, incoming edges' [h|als] rows are
    fetched with dma_gather by src (4 SWDGE queues over 25088-row chunks so
    indices fit int16; trailing padding uses idx=-1 which the gather skips),
    ad rows are fetched with one dma_gather by dst from the local table,
    z = als+ad -> LeakyReLU+Exp on ScalarE, msg = h*p on DVE, and the segment
    softmax numerator/denominator accumulate with host-precomputed one-hot
    matmuls on TensorE (O2 streamed from DRAM, layer-independent).
  - flush: out = num/denom, ELU via ScalarE Relu/Exp + one DVE op, store.
Pooling/FC run replicated on every core from the AllGathered final layer.

Self-contained: accepts the FULL inputs, returns the FULL [512, 6] output.
Falls back to a pure-numpy implementation if the device path fails.
"""
import os
import numpy as np

H = 8
N_GRAPHS = 512
NPAD = 100352            # 784 * 128
NCORES = 8
NCHUNK = 4
CHUNK = NPAD // NCHUNK   # 25088
WPC = NPAD // 128 // NCORES   # 98 windows (node tiles) per core
NODES_PC = NPAD // NCORES     # 12544
POOL_BLOCK = 1024
ROW = 256                # strided node-table row (bf16 cols); 512B per row
HROW = 136               # used cols: h(128) + als(8)

_LAST_EXEC_NS = None     # set by the device path; read by test harness


# ----------------------------------------------------------------------------
# CPU prep
# ----------------------------------------------------------------------------

def _build_edge_streams(src, dst):
    """Group edges (dst-sorted) by (128-dst-window, src-chunk); pad each group
    to a common G slots. Returns per-global-window arrays:
      idx_src [784,4,G] int16  chunk-local src; padding = -1 (gather skips)
      dst_loc [784,4,G] int32  window-local dst slot (w_local*128+drel built
                               later per core); padding = -1 marker
      drel    [784,4,G] int16  dst - window_base; padding = -1
    plus G."""
    order = np.argsort(dst, kind='stable')
    src_s = src[order].astype(np.int64)
    dst_s = dst[order].astype(np.int64)
    win = dst_s >> 7
    chunk = src_s // CHUNK
    order2 = np.lexsort((chunk, win))
    src_s = src_s[order2]
    dst_s = dst_s[order2]
    win = win[order2]
    chunk = chunk[order2]
    gidx = win * NCHUNK + chunk
    counts = np.bincount(gidx, minlength=784 * NCHUNK)
    G = int(counts.max())
    G = -(-G // 128) * 128                     # output slots need 128-alignment
    starts = np.zeros(784 * NCHUNK, dtype=np.int64)
    np.cumsum(counts[:-1], out=starts[1:])
    n_e = src_s.shape[0]
    slot_of_edge = np.repeat(starts, counts)   # group start per edge
    within = np.arange(n_e) - slot_of_edge
    flat_slot = gidx * G + within
    idx_src = np.full(784 * NCHUNK * G, -1, dtype=np.int16)
    drel = np.full(784 * NCHUNK * G, -1, dtype=np.int16)
    idx_src[flat_slot] = (src_s - chunk * CHUNK).astype(np.int16)
    drel[flat_slot] = (dst_s - (win << 7)).astype(np.int16)
    return idx_src.reshape(784, NCHUNK, G), drel.reshape(784, NCHUNK, G), G


def _graph_segments(batch):
    """Per graph: list of (block, col0, width) segments within POOL_BLOCK node
    blocks, plus counts."""
    cnt = np.bincount(batch, minlength=N_GRAPHS).astype(np.int64)
    bstarts = np.zeros(N_GRAPHS, dtype=np.int64)
    np.cumsum(cnt[:-1], out=bstarts[1:])
    segs = []
    for g in range(N_GRAPHS):
        s, e = int(bstarts[g]), int(bstarts[g] + cnt[g])
        out = []
        while s < e:
            blk = s // POOL_BLOCK
            e_blk = min(e, (blk + 1) * POOL_BLOCK)
            out.append((blk, s - blk * POOL_BLOCK, e_blk - s))
            s = e_blk
        segs.append(out)
    return segs, cnt


def _wrap16(a):
    """[n] int16 -> [128, n//16] in the dma_gather wrapped+replicated format."""
    n = a.shape[0]
    aw = a.reshape(n // 16, 16)
    aw = np.moveaxis(aw, 1, 0).reshape(16, n // 16)
    return np.tile(np.ascontiguousarray(aw), (8, 1))


def _prep(x, edge_index, batch, Ws, Ads, Ass, bs, fcW, fcb):
    """All CPU-side preparation. Returns dict of per-core inputs + constants."""
    import ml_dtypes
    bf16 = ml_dtypes.bfloat16

    n = x.shape[0]
    loops = np.arange(n, dtype=np.int64)
    src = np.concatenate([np.asarray(edge_index[0], dtype=np.int64), loops])
    dst = np.concatenate([np.asarray(edge_index[1], dtype=np.int64), loops])
    idx_src, drel, G = _build_edge_streams(src, dst)
    T = NCHUNK * G // 128

    idx_src_w = []       # per-core [128, WPC*NCHUNK*G/16] int16
    idx_dst_w = []       # per-core [128, WPC*NCHUNK*G/16] int16
    O2_w = []            # per-core [WPC*128, T*128] bf16
    for cidx in range(NCORES):
        a = idx_src[cidx * WPC:(cidx + 1) * WPC]        # [98, 4, G]
        idx_src_w.append(_wrap16(a.reshape(-1)))
        d = drel[cidx * WPC:(cidx + 1) * WPC]           # [98, 4, G]
        # dst slot within shard = w_local*128 + drel; pads -> dummy row 0
        wloc = np.arange(WPC, dtype=np.int32)[:, None, None]
        dloc = wloc * 128 + d.astype(np.int32)
        dloc[d < 0] = 0
        idx_dst_w.append(_wrap16(dloc.astype(np.int16).reshape(-1)))
        # dense one-hot O2: [w*128+e, t*128+drel] = 1 for real slots
        O2 = np.zeros((WPC * 128, T * 128), dtype=bf16)
        wi, ci, gi = np.nonzero(d >= 0)
        slot = ci * G + gi                               # slot within window
        t = slot >> 7
        e = slot & 127
        O2[wi * 128 + e, t * 128 + d[wi, ci, gi]] = 1.0
        O2_w.append(O2)

    # layer dims: (cin_pad, c, hout)
    dims = [(32, 8, 64), (128, 16, 128), (128, 16, 128), (128, 16, 128)]
    layers = []
    for li, (cin, c, hout) in enumerate(dims):
        Wp = np.zeros((cin, 128), dtype=np.float32)
        Wr = np.asarray(Ws[li], dtype=np.float32)
        Wp[:Wr.shape[0], :Wr.shape[1]] = Wr
        Ad = np.asarray(Ads[li], dtype=np.float32)
        As = np.asarray(Ass[li], dtype=np.float32)
        WAd = np.zeros((cin, 8), dtype=np.float32)
        WAs = np.zeros((cin, 8), dtype=np.float32)
        for h in range(H):
            WAd[:, h] = Wp[:, h * c:(h + 1) * c] @ Ad[h]
            WAs[:, h] = Wp[:, h * c:(h + 1) * c] @ As[h]
        b = np.zeros(128, dtype=np.float32)
        b[:hout] = np.asarray(bs[li], dtype=np.float32)
        layers.append(dict(
            cin=cin, c=c, hout=hout,
            W=Wp.astype(bf16),
            WAd=WAd.astype(bf16),
            WAs=WAs.astype(bf16),
            bias=np.tile(b[None, :], (128, 1)).astype(np.float32),
            has_bias=bool(np.any(b != 0)),
        ))

    # x shards, transposed [32, 12544] bf16
    xp = np.zeros((NPAD, 32), dtype=np.float32)
    xp[:n, :x.shape[1]] = np.asarray(x, dtype=np.float32)
    x0T = [np.ascontiguousarray(xp[cidx * NODES_PC:(cidx + 1) * NODES_PC].T).astype(bf16)
           for cidx in range(NCORES)]

    segs, cnt = _graph_segments(np.asarray(batch, dtype=np.int64))
    rcp = np.where(cnt > 0, 1.0 / np.maximum(cnt, 1), 0.0).astype(np.float32)
    rcp_t = np.zeros((128, 4), dtype=np.float32)
    rcp_t[:, :] = rcp.reshape(4, 128).T

    fcW = np.asarray(fcW, dtype=np.float32)
    consts = dict(
        fcWm=fcW[:128].astype(bf16),
        fcWx=fcW[128:].astype(bf16),
        fcb=np.tile(np.asarray(fcb, dtype=np.float32)[None, :], (128, 1)),
        rcp_t=rcp_t,
    )
    return dict(G=G, T=T, layers=layers, idx_src_w=idx_src_w,
                idx_dst_w=idx_dst_w, O2_w=O2_w, x0T=x0T,
                segs=segs, consts=consts)


# ----------------------------------------------------------------------------
# Device program
# ----------------------------------------------------------------------------

def _build_program(p):
    import concourse.bass as bass
    import concourse.mybir as mybir
    import concourse.tile as tile
    from concourse import bacc

    G, T = p['G'], p['T']
    layers = p['layers']
    segs = p['segs']
    f32 = mybir.dt.float32
    bf = mybir.dt.bfloat16
    AX = mybir.AxisListType.X
    OP = mybir.AluOpType
    ACT = mybir.ActivationFunctionType

    nc = bacc.Bacc("TRN2", target_bir_lowering=False, debug=False,
                   num_devices=NCORES, num_swdge_queues=4)

    GI = G // 16              # idx cols per (w,cc) group
    G128 = G // 128           # gather out tiles per chunk

    # ---- I/O ----
    t_x0T = nc.dram_tensor("x0T", [32, NODES_PC], bf, kind="ExternalInput")
    t_isrc = nc.dram_tensor("isrc", [128, WPC * NCHUNK * GI], mybir.dt.int16,
                            kind="ExternalInput")
    t_idst = nc.dram_tensor("idst", [128, WPC * NCHUNK * GI], mybir.dt.int16,
                            kind="ExternalInput")
    t_O2 = nc.dram_tensor("O2", [WPC * 128, T * 128], bf, kind="ExternalInput")
    t_W, t_WAd, t_WAs, t_bias = [], [], [], []
    for li, L in enumerate(layers):
        t_W.append(nc.dram_tensor(f"W{li}", [L['cin'], 128], bf, kind="ExternalInput"))
        t_WAd.append(nc.dram_tensor(f"WAd{li}", [L['cin'], 8], bf, kind="ExternalInput"))
        t_WAs.append(nc.dram_tensor(f"WAs{li}", [L['cin'], 8], bf, kind="ExternalInput"))
        t_bias.append(nc.dram_tensor(f"bias{li}", [128, 128], f32, kind="ExternalInput"))
    t_fcWm = nc.dram_tensor("fcWm", [128, 6], bf, kind="ExternalInput")
    t_fcWx = nc.dram_tensor("fcWx", [128, 6], bf, kind="ExternalInput")
    t_fcb = nc.dram_tensor("fcb", [128, 6], f32, kind="ExternalInput")
    t_rcp = nc.dram_tensor("rcp", [128, 4], f32, kind="ExternalInput")
    t_out = nc.dram_tensor("out", [N_GRAPHS, 6], f32, kind="ExternalOutput")

    with tile.TileContext(nc) as tc:
        with (
            tc.tile_pool(name="const", bufs=1) as cpool,
            tc.tile_pool(name="persist", bufs=1) as ppool,
            tc.tile_pool(name="work", bufs=3) as wpool,
            tc.tile_pool(name="gath", bufs=2) as gpool,
            tc.tile_pool(name="psum", bufs=2, space="PSUM") as pspool,
            tc.tile_pool(name="psw", bufs=2, space="PSUM") as pswin,
            tc.tile_pool(name="dram", bufs=1, space="DRAM") as dpool,
        ):
            # ---- load constants ----
            def cload(t, shape, dtyp):
                s = cpool.tile(shape, dtyp, tag=t.name)
                nc.sync.dma_start(out=s[:], in_=t[:, :])
                return s
            fcWm = cload(t_fcWm, [128, 6], bf)
            fcWx = cload(t_fcWx, [128, 6], bf)
            fcb = cload(t_fcb, [128, 6], f32)
            rcp_t = cload(t_rcp, [128, 4], f32)
            Wsb, WAdsb, WAssb, biases = [], [], [], []
            for li, L in enumerate(layers):
                Wsb.append(cload(t_W[li], [L['cin'], 128], bf))
                WAdsb.append(cload(t_WAd[li], [L['cin'], 8], bf))
                WAssb.append(cload(t_WAs[li], [L['cin'], 8], bf))
                biases.append(cload(t_bias[li], [128, 128], f32))
            isrc_sb = ppool.tile([128, WPC * NCHUNK * GI], mybir.dt.int16)
            nc.sync.dma_start(out=isrc_sb[:], in_=t_isrc[:, :])
            idst_sb = ppool.tile([128, WPC * NCHUNK * GI], mybir.dt.int16)
            nc.sync.dma_start(out=idst_sb[:], in_=t_idst[:, :])
            x0T_sb = ppool.tile([32, NODES_PC], bf)
            nc.sync.dma_start(out=x0T_sb[:], in_=t_x0T[:, :])

            zero64 = cpool.tile([128, 64], bf, tag="zero64")
            nc.vector.memset(zero64[:], 0.0)

            # gather landing buffers (manually double-buffered): memset once so
            # skipped (padding) slots hold finite stale data instead of garbage
            hg_bufs = [ppool.tile([128, T, ROW], bf, name=f"hgbuf{i}")
                       for i in range(2)]
            ad_bufs = [ppool.tile([128, T, 128], bf, name=f"adbuf{i}")
                       for i in range(2)]
            for b_ in hg_bufs + ad_bufs:
                nc.vector.memset(b_[:], 0.0)

            out_own = None  # [NODES_PC, 128] DRAM, own shard of prev layer out
            for li, L in enumerate(layers):
                cin, c, hout = L['cin'], L['c'], L['hout']
                # ---- node phase: h | als rows + ad table for own shard ----
                h_shard = dpool.tile([NODES_PC, HROW], bf, tag="h_shard")
                att_shard = dpool.tile([NODES_PC, 128], bf, tag="att_shard")
                h_tbl_c = dpool.tile([NPAD, HROW], bf, tag="h_tbl_c")
                h_tbl = dpool.tile([NPAD, ROW], bf, tag="h_tbl")
                for nt in range(WPC):
                    if li == 0:
                        xT = x0T_sb[:cin, nt * 128:(nt + 1) * 128]
                    else:
                        xTt = wpool.tile([cin, 128], bf, tag="xT")
                        nc.sync.dma_start(
                            out=xTt[:],
                            in_=out_own[nt * 128:(nt + 1) * 128, :],
                            transpose=True)
                        xT = xTt[:]
                    nd_ps = pspool.tile([128, 144], f32, tag="nd_ps", space="PSUM")
                    nc.tensor.matmul(nd_ps[:, :128], xT, Wsb[li][:], start=True, stop=True)
                    nc.tensor.matmul(nd_ps[:, 128:136], xT, WAssb[li][:], start=True, stop=True)
                    nc.tensor.matmul(nd_ps[:, 136:144], xT, WAdsb[li][:], start=True, stop=True)
                    h_row = wpool.tile([128, HROW], bf, tag="h_row")
                    nc.vector.tensor_copy(h_row[:], nd_ps[:, :HROW])
                    ad_row = wpool.tile([128, 8], bf, tag="ad_row")
                    nc.vector.tensor_copy(ad_row[:], nd_ps[:, 136:144])
                    nc.sync.dma_start(out=h_shard[nt * 128:(nt + 1) * 128, :], in_=h_row[:])
                    nc.sync.dma_start(out=att_shard[nt * 128:(nt + 1) * 128, 0:8],
                                      in_=ad_row[:])
                # AllGather compact rows, then expand to 512B-strided table
                nc.gpsimd.collective_compute(
                    "AllGather", OP.bypass,
                    replica_groups=[list(range(NCORES))],
                    ins=[h_shard.opt()], outs=[h_tbl_c.opt()])
                for xc in range(4):
                    r0, r1 = xc * (NPAD // 4), (xc + 1) * (NPAD // 4)
                    nc.sync.dma_start(out=h_tbl[r0:r1, 0:HROW], in_=h_tbl_c[r0:r1, :])

                # ---- edge phase ----
                out_new = dpool.tile([NODES_PC, 128], bf, tag="out_own")
                for w in range(WPC):
                    hg_win = hg_bufs[w % 2]
                    ad_win = ad_bufs[w % 2]
                    for cc in range(NCHUNK):
                        nc.gpsimd.dma_gather(
                            out_ap=hg_win[:, cc * G128:(cc + 1) * G128, :],
                            in_ap=h_tbl[cc * CHUNK:(cc + 1) * CHUNK, :],
                            idxs_ap=isrc_sb[:, (w * NCHUNK + cc) * GI:
                                            (w * NCHUNK + cc + 1) * GI],
                            num_idxs=G, num_idxs_reg=G, elem_size=ROW,
                            single_packet=False, queue_num=cc)
                    nc.gpsimd.dma_gather(
                        out_ap=ad_win[:, :, :],
                        in_ap=att_shard[:, :],
                        idxs_ap=idst_sb[:, w * NCHUNK * GI:(w + 1) * NCHUNK * GI],
                        num_idxs=NCHUNK * G, num_idxs_reg=NCHUNK * G, elem_size=128,
                        single_packet=False, queue_num=w % 4)
                    O2w = gpool.tile([128, T * 128], bf, tag="O2w")
                    nc.sync.dma_start(out=O2w[:],
                                      in_=t_O2[w * 128:(w + 1) * 128, :])
                    ps_win = pswin.tile([128, 136], f32, tag="ps_win", space="PSUM")
                    for tp in range(T // 2):
                        t0 = 2 * tp
                        z2 = wpool.tile([128, 2, 8], f32, tag="z")
                        nc.vector.tensor_add(z2[:], hg_win[:, t0:t0 + 2, 128:136],
                                             ad_win[:, t0:t0 + 2, 0:8])
                        zl = wpool.tile([128, 2, 8], f32, tag="zl")
                        nc.scalar.activation(zl[:], z2[:], ACT.Lrelu, alpha=0.2)
                        msg2 = wpool.tile([128, 2, 136], bf, tag="msg")
                        nc.scalar.activation(msg2[:, :, 128:136], zl[:], ACT.Exp)
                        nc.vector.tensor_mul(
                            msg2[:, :, :hout].rearrange("p o (h c) -> p o h c", c=c),
                            hg_win[:, t0:t0 + 2, :hout]
                            .rearrange("p o (h c) -> p o h c", c=c),
                            msg2[:, :, 128:136].rearrange("p o (h c) -> p o h c", c=1)
                            .to_broadcast([128, 2, 8, c]))
                        for t in (0, 1):
                            first = (t0 + t == 0)
                            last = (t0 + t == T - 1)
                            O2t = O2w[:, (t0 + t) * 128:(t0 + t + 1) * 128]
                            if hout == 128:
                                nc.tensor.matmul(ps_win[:, :136], O2t,
                                                 msg2[:, t, :],
                                                 start=first, stop=last)
                            else:
                                nc.tensor.matmul(ps_win[:, :hout], O2t,
                                                 msg2[:, t, :hout],
                                                 start=first, stop=last)
                                nc.tensor.matmul(ps_win[:, 128:136], O2t,
                                                 msg2[:, t, 128:136],
                                                 start=first, stop=last)
                    # ---- flush window ----
                    den = wpool.tile([128, 8], f32, tag="den")
                    nc.vector.tensor_scalar_max(den[:], ps_win[:, 128:136], 1e-20)
                    rcpd = wpool.tile([128, 8], f32, tag="rcpd")
                    nc.vector.reciprocal(rcpd[:], den[:])
                    o1 = wpool.tile([128, hout], f32, tag="o1")
                    nc.vector.tensor_mul(
                        o1[:].rearrange("p (h c) -> p h c", c=c),
                        ps_win[:, :hout].rearrange("p (h c) -> p h c", c=c),
                        rcpd[:].rearrange("p (h o) -> p h o", o=1).to_broadcast([128, 8, c]))
                    if L['has_bias']:
                        nc.vector.tensor_add(o1[:], o1[:], biases[li][:, :hout])
                    # ELU on ScalarE: relu(x) + exp(-relu(-x)) - 1
                    pos = wpool.tile([128, hout], bf, tag="pos")
                    nc.scalar.activation(pos[:], o1[:], ACT.Relu)
                    nneg = wpool.tile([128, hout], f32, tag="nneg")
                    nc.scalar.activation(nneg[:], o1[:], ACT.Relu, scale=-1.0)
                    en = wpool.tile([128, hout], f32, tag="en")
                    nc.scalar.activation(en[:], nneg[:], ACT.Exp, scale=-1.0)
                    ob = wpool.tile([128, hout], bf, tag="ob")
                    nc.vector.scalar_tensor_tensor(ob[:], en[:], -1.0, pos[:],
                                                   op0=OP.add, op1=OP.add)
                    nc.sync.dma_start(out=out_new[w * 128:(w + 1) * 128, :hout], in_=ob[:])
                    if hout < 128:
                        nc.sync.dma_start(out=out_new[w * 128:(w + 1) * 128, hout:],
                                          in_=zero64[:])
                out_own = out_new

            # ---- final AllGather of out4 + pooling ----
            out4 = dpool.tile([NPAD, 128], bf, tag="out4")
            nc.gpsimd.collective_compute(
                "AllGather", OP.bypass,
                replica_groups=[list(range(NCORES))],
                ins=[out_own.opt()], outs=[out4.opt()])

            sumsT = ppool.tile([128, N_GRAPHS], f32)
            mxT = ppool.tile([128, N_GRAPHS], f32)
            nc.vector.memset(sumsT[:], 0.0)
            nc.vector.memset(mxT[:], 0.0)
            blk_tiles = {}
            needed_blocks = sorted({s[0] for gs in segs for s in gs})
            for blk in needed_blocks:
                bt = wpool.tile([128, POOL_BLOCK], bf, tag="poolblk")
                nc.sync.dma_start(out=bt[:],
                                  in_=out4[blk * POOL_BLOCK:(blk + 1) * POOL_BLOCK, :],
                                  transpose=True)
                blk_tiles[blk] = bt
                for g, gs in enumerate(segs):
                    if not gs or gs[-1][0] != blk:
                        continue
                    if len(gs) == 1:
                        b0, c0, wd = gs[0]
                        nc.vector.reduce_sum(sumsT[:, g:g + 1],
                                             blk_tiles[b0][:, c0:c0 + wd], axis=AX)
                        nc.vector.reduce_max(mxT[:, g:g + 1],
                                             blk_tiles[b0][:, c0:c0 + wd], axis=AX)
                    else:
                        tmp_s = wpool.tile([128, 1], f32, tag="tmp_s")
                        tmp_m = wpool.tile([128, 1], f32, tag="tmp_m")
                        first = True
                        for (b0, c0, wd) in gs:
                            if first:
                                nc.vector.reduce_sum(sumsT[:, g:g + 1],
                                                     blk_tiles[b0][:, c0:c0 + wd], axis=AX)
                                nc.vector.reduce_max(mxT[:, g:g + 1],
                                                     blk_tiles[b0][:, c0:c0 + wd], axis=AX)
                                first = False
                            else:
                                nc.vector.reduce_sum(tmp_s[:],
                                                     blk_tiles[b0][:, c0:c0 + wd], axis=AX)
                                nc.vector.reduce_max(tmp_m[:],
                                                     blk_tiles[b0][:, c0:c0 + wd], axis=AX)
                                nc.vector.tensor_add(sumsT[:, g:g + 1],
                                                     sumsT[:, g:g + 1], tmp_s[:])
                                nc.vector.tensor_max(mxT[:, g:g + 1],
                                                     mxT[:, g:g + 1], tmp_m[:])
            sumsTb = ppool.tile([128, N_GRAPHS], bf)
            mxTb = ppool.tile([128, N_GRAPHS], bf)
            nc.vector.tensor_copy(sumsTb[:], sumsT[:])
            nc.vector.tensor_copy(mxTb[:], mxT[:])

            for gt in range(N_GRAPHS // 128):
                p1 = pspool.tile([128, 6], f32, tag="p1", space="PSUM")
                p2 = pspool.tile([128, 6], f32, tag="p2", space="PSUM")
                nc.tensor.matmul(p1[:], sumsTb[:, gt * 128:(gt + 1) * 128], fcWm[:],
                                 start=True, stop=True)
                nc.tensor.matmul(p2[:], mxTb[:, gt * 128:(gt + 1) * 128], fcWx[:],
                                 start=True, stop=True)
                zz = wpool.tile([128, 6], f32, tag="zz")
                nc.vector.tensor_mul(zz[:], p1[:],
                                     rcp_t[:, gt:gt + 1].to_broadcast([128, 6]))
                nc.vector.tensor_add(zz[:], zz[:], p2[:])
                nc.vector.tensor_add(zz[:], zz[:], fcb[:])
                m6 = wpool.tile([128, 1], f32, tag="m6")
                nc.vector.reduce_max(m6[:], zz[:], axis=AX)
                nc.vector.tensor_sub(zz[:], zz[:], m6[:].to_broadcast([128, 6]))
                e6 = wpool.tile([128, 6], f32, tag="e6")
                nc.scalar.activation(e6[:], zz[:], ACT.Exp)
                s6 = wpool.tile([128, 1], f32, tag="s6")
                nc.vector.reduce_sum(s6[:], e6[:], axis=AX)
                nc.scalar.activation(s6[:], s6[:], ACT.Ln)
                nc.vector.tensor_sub(zz[:], zz[:], s6[:].to_broadcast([128, 6]))
                nc.sync.dma_start(out=t_out[gt * 128:(gt + 1) * 128, :], in_=zz[:])
    nc.compile()
    return nc


def _kernel_trn(x, edge_index, batch,
                W1, a1s, a1d, b1, W2, a2s, a2d, b2,
                W3, a3s, a3d, b3, W4, a4s, a4d, b4, fcW, fcb):
    global _LAST_EXEC_NS
    from concourse.bass_utils import run_bass_kernel_spmd

    p = _prep(np.asarray(x), np.asarray(edge_index), np.asarray(batch),
              [W1, W2, W3, W4], [a1d, a2d, a3d, a4d], [a1s, a2s, a3s, a4s],
              [b1, b2, b3, b4], fcW, fcb)
    nc = _build_program(p)

    in_maps = []
    for cidx in range(NCORES):
        m = dict(
            x0T=p['x0T'][cidx], isrc=p['idx_src_w'][cidx],
            idst=p['idx_dst_w'][cidx], O2=p['O2_w'][cidx],
            fcWm=p['consts']['fcWm'], fcWx=p['consts']['fcWx'],
            fcb=p['consts']['fcb'], rcp=p['consts']['rcp_t'],
        )
        for li, L in enumerate(p['layers']):
            m[f"W{li}"] = L['W']
            m[f"WAd{li}"] = L['WAd']
            m[f"WAs{li}"] = L['WAs']
            m[f"bias{li}"] = L['bias']
        in_maps.append(m)

    trace = bool(int(os.environ.get("GAT_TRACE", "0")))
    tdir = os.environ.get("GAT_TRACE_DIR") or None
    res = run_bass_kernel_spmd(nc, in_maps, core_ids=list(range(NCORES)),
                               trace=trace, tmpdir=tdir)
    _LAST_EXEC_NS = res.exec_time_ns
    globals()['_LAST_RES'] = res
    return np.asarray(res.results[0]["out"], dtype=np.float32)


# ----------------------------------------------------------------------------
# numpy fallback (previous baseline)
# ----------------------------------------------------------------------------

def _leaky_relu(v, slope=0.2):
    return np.where(v >= 0, v, slope * v)


def _elu(v):
    return np.where(v > 0, v, np.expm1(np.minimum(v, 0.0)))


def _gat_layer_np(x, W, a_src, a_dst, b, src_s, dst_s, starts, n, c):
    h = (x @ W).reshape(n, H, c)
    al_s = np.einsum('nhc,hc->nh', h, a_src)
    al_d = np.einsum('nhc,hc->nh', h, a_dst)
    e = _leaky_relu(al_s[src_s] + al_d[dst_s])
    e_max = np.maximum.reduceat(e, starts, axis=0)
    ex = np.exp(e - e_max[dst_s])
    denom = np.add.reduceat(ex, starts, axis=0)
    alpha = ex / denom[dst_s]
    msg = h[src_s]
    msg *= alpha[:, :, None]
    out = np.add.reduceat(msg.reshape(-1, H * c), starts, axis=0)
    return out + b


def _kernel_numpy(x, edge_index, batch,
                  W1, a1s, a1d, b1, W2, a2s, a2d, b2,
                  W3, a3s, a3d, b3, W4, a4s, a4d, b4, fcW, fcb):
    x = np.asarray(x, dtype=np.float32)
    edge_index = np.asarray(edge_index)
    batch = np.asarray(batch)
    n = x.shape[0]
    loops = np.arange(n, dtype=edge_index.dtype)
    src = np.concatenate([edge_index[0], loops])
    dst = np.concatenate([edge_index[1], loops])
    order = np.argsort(dst, kind='stable')
    src_s = src[order]
    dst_s = dst[order]
    counts = np.bincount(dst, minlength=n)
    starts = np.zeros(n, dtype=np.int64)
    np.cumsum(counts[:-1], out=starts[1:])
    x = _elu(_gat_layer_np(x, W1, a1s, a1d, b1, src_s, dst_s, starts, n, 8))
    x = _elu(_gat_layer_np(x, W2, a2s, a2d, b2, src_s, dst_s, starts, n, 16))
    x = _elu(_gat_layer_np(x, W3, a3s, a3d, b3, src_s, dst_s, starts, n, 16))
    x = _elu(_gat_layer_np(x, W4, a4s, a4d, b4, src_s, dst_s, starts, n, 16))
    cnt = np.bincount(batch, minlength=N_GRAPHS)
    nz = cnt > 0
    bstarts = np.zeros(N_GRAPHS, dtype=np.int64)
    np.cumsum(cnt[:-1], out=bstarts[1:])
    f = x.shape[1]
    mean = np.zeros((N_GRAPHS, f), dtype=np.float32)
    mx = np.zeros((N_GRAPHS, f), dtype=np.float32)
    nz_starts = bstarts[nz]
    mean[nz] = np.add.reduceat(x, nz_starts, axis=0) / cnt[nz, None]
    mx[nz] = np.maximum.reduceat(x, nz_starts, axis=0)
    feat = np.concatenate([mean, mx], axis=1)
    z = feat @ fcW + fcb
    z -= z.max(axis=1, keepdims=True)
    z -= np.log(np.exp(z).sum(axis=1, keepdims=True))
    return z.astype(np.float32)


def kernel(**inputs):
    if not int(os.environ.get("GAT_FORCE_NUMPY", "0")):
        try:
            return _kernel_trn(**inputs)
        except Exception:
            import traceback
            traceback.print_exc()
    return _kernel_numpy(**inputs)


# revision 11
# speedup vs baseline: 1.1380x; 1.1380x over previous
"""GatNet on 8 Trainium2 NeuronCores (Bass/Tile).

4-layer GAT (8 heads) + mean/max graph pooling + FC + log_softmax.

Distribution: edges partitioned by destination node across the 8 cores
(each core owns a contiguous range of 12544 destination nodes and all edges
into them). Per layer:
  - node phase (sharded): h = x @ W, als = x @ (W a_src), ad = x @ (W a_dst)
    for own nodes; h+als go into a 136-col row table that is AllGathered and
    expanded to 512B-strided rows (dma_gather elem_size must be a multiple of
    256B); ad goes into a compact local per-shard table (dst side is local).
  - edge phase: per 128-dst-node window, incoming edges' [h|als] rows are
    fetched with dma_gather by src (4 SWDGE queues over 25088-row chunks so
    indices fit int16; trailing padding uses idx=-1 which the gather skips),
    ad rows are fetched with one dma_gather by dst from the local table,
    z = als+ad -> LeakyReLU+Exp on ScalarE, msg = h*p on DVE, and the segment
    softmax numerator/denominator accumulate with host-precomputed one-hot
    matmuls on TensorE (O2 streamed from DRAM, layer-independent).
  - flush: out = num/denom, ELU via ScalarE Relu/Exp + one DVE op, store.
Pooling/FC run replicated on every core from the AllGathered final layer.

Self-contained: accepts the FULL inputs, returns the FULL [512, 6] output.
Falls back to a pure-numpy implementation if the device path fails.
"""
import os
import numpy as np

H = 8
N_GRAPHS = 512
NPAD = 100352            # 784 * 128
NCORES = 8
NCHUNK = 4
CHUNK = NPAD // NCHUNK   # 25088
WPC = NPAD // 128 // NCORES   # 98 windows (node tiles) per core
NODES_PC = NPAD // NCORES     # 12544
POOL_BLOCK = 1024
ROW = 256                # strided node-table row (bf16 cols); 512B per row
HROW = 136               # used cols: h(128) + als(8)

_LAST_EXEC_NS = None     # set by the device path; read by test harness


# ----------------------------------------------------------------------------
# CPU prep
# ----------------------------------------------------------------------------

def _build_edge_streams(src, dst):
    """Group edges (dst-sorted) by (128-dst-window, src-chunk); pad each group
    to a common G slots. Returns per-global-window arrays:
      idx_src [784,4,G] int16  chunk-local src; padding = -1 (gather skips)
      dst_loc [784,4,G] int32  window-local dst slot (w_local*128+drel built
                               later per core); padding = -1 marker
      drel    [784,4,G] int16  dst - window_base; padding = -1
    plus G."""
    order = np.argsort(dst, kind='stable')
    src_s = src[order].astype(np.int64)
    dst_s = dst[order].astype(np.int64)
    win = dst_s >> 7
    chunk = src_s // CHUNK
    order2 = np.lexsort((chunk, win))
    src_s = src_s[order2]
    dst_s = dst_s[order2]
    win = win[order2]
    chunk = chunk[order2]
    gidx = win * NCHUNK + chunk
    counts = np.bincount(gidx, minlength=784 * NCHUNK)
    G = int(counts.max())
    G = -(-G // 128) * 128                     # output slots need 128-alignment
    starts = np.zeros(784 * NCHUNK, dtype=np.int64)
    np.cumsum(counts[:-1], out=starts[1:])
    n_e = src_s.shape[0]
    slot_of_edge = np.repeat(starts, counts)   # group start per edge
    within = np.arange(n_e) - slot_of_edge
    flat_slot = gidx * G + within
    idx_src = np.full(784 * NCHUNK * G, -1, dtype=np.int16)
    drel = np.full(784 * NCHUNK * G, -1, dtype=np.int16)
    idx_src[flat_slot] = (src_s - chunk * CHUNK).astype(np.int16)
    drel[flat_slot] = (dst_s - (win << 7)).astype(np.int16)
    return idx_src.reshape(784, NCHUNK, G), drel.reshape(784, NCHUNK, G), G


def _graph_segments(batch):
    """Per graph: list of (block, col0, width) segments within POOL_BLOCK node
    blocks, plus counts."""
    cnt = np.bincount(batch, minlength=N_GRAPHS).astype(np.int64)
    bstarts = np.zeros(N_GRAPHS, dtype=np.int64)
    np.cumsum(cnt[:-1], out=bstarts[1:])
    segs = []
    for g in range(N_GRAPHS):
        s, e = int(bstarts[g]), int(bstarts[g] + cnt[g])
        out = []
        while s < e:
            blk = s // POOL_BLOCK
            e_blk = min(e, (blk + 1) * POOL_BLOCK)
            out.append((blk, s - blk * POOL_BLOCK, e_blk - s))
            s = e_blk
        segs.append(out)
    return segs, cnt


def _wrap16(a):
    """[n] int16 -> [128, n//16] in the dma_gather wrapped+replicated format."""
    n = a.shape[0]
    aw = a.reshape(n // 16, 16)
    aw = np.moveaxis(aw, 1, 0).reshape(16, n // 16)
    return np.tile(np.ascontiguousarray(aw), (8, 1))


def _prep(x, edge_index, batch, Ws, Ads, Ass, bs, fcW, fcb):
    """All CPU-side preparation. Returns dict of per-core inputs + constants."""
    import ml_dtypes
    bf16 = ml_dtypes.bfloat16

    n = x.shape[0]
    loops = np.arange(n, dtype=np.int64)
    src = np.concatenate([np.asarray(edge_index[0], dtype=np.int64), loops])
    dst = np.concatenate([np.asarray(edge_index[1], dtype=np.int64), loops])
    idx_src, drel, G = _build_edge_streams(src, dst)
    T = NCHUNK * G // 128

    idx_src_w = []       # per-core [128, WPC*NCHUNK*G/16] int16
    idx_dst_w = []       # per-core [128, WPC*NCHUNK*G/16] int16
    O2_w = []            # per-core [WPC*128, T*128] bf16
    for cidx in range(NCORES):
        a = idx_src[cidx * WPC:(cidx + 1) * WPC]        # [98, 4, G]
        idx_src_w.append(_wrap16(a.reshape(-1)))
        d = drel[cidx * WPC:(cidx + 1) * WPC]           # [98, 4, G]
        # dst slot within shard = w_local*128 + drel; pads -> dummy row 0
        wloc = np.arange(WPC, dtype=np.int32)[:, None, None]
        dloc = wloc * 128 + d.astype(np.int32)
        dloc[d < 0] = 0
        idx_dst_w.append(_wrap16(dloc.astype(np.int16).reshape(-1)))
        # drel per slot in [128, WPC*T] layout (partition = slot-within-tile);
        # pads are -1 -> all-zero one-hot column
        dv = d.reshape(WPC, T, 128)
        dv = np.moveaxis(dv, 2, 0).reshape(128, WPC * T)
        O2_w.append(np.ascontiguousarray(dv).astype(bf16))

    # layer dims: (cin_pad, c, hout)
    dims = [(32, 8, 64), (128, 16, 128), (128, 16, 128), (128, 16, 128)]
    layers = []
    for li, (cin, c, hout) in enumerate(dims):
        Wp = np.zeros((cin, 128), dtype=np.float32)
        Wr = np.asarray(Ws[li], dtype=np.float32)
        Wp[:Wr.shape[0], :Wr.shape[1]] = Wr
        Ad = np.asarray(Ads[li], dtype=np.float32)
        As = np.asarray(Ass[li], dtype=np.float32)
        WAd = np.zeros((cin, 8), dtype=np.float32)
        WAs = np.zeros((cin, 8), dtype=np.float32)
        for h in range(H):
            WAd[:, h] = Wp[:, h * c:(h + 1) * c] @ Ad[h]
            WAs[:, h] = Wp[:, h * c:(h + 1) * c] @ As[h]
        b = np.zeros(128, dtype=np.float32)
        b[:hout] = np.asarray(bs[li], dtype=np.float32)
        layers.append(dict(
            cin=cin, c=c, hout=hout,
            W=Wp.astype(bf16),
            WAd=WAd.astype(bf16),
            WAs=WAs.astype(bf16),
            bias=np.tile(b[None, :], (128, 1)).astype(np.float32),
            has_bias=bool(np.any(b != 0)),
        ))

    # x shards, transposed [32, 12544] bf16
    xp = np.zeros((NPAD, 32), dtype=np.float32)
    xp[:n, :x.shape[1]] = np.asarray(x, dtype=np.float32)
    x0T = [np.ascontiguousarray(xp[cidx * NODES_PC:(cidx + 1) * NODES_PC].T).astype(bf16)
           for cidx in range(NCORES)]

    segs, cnt = _graph_segments(np.asarray(batch, dtype=np.int64))
    rcp = np.where(cnt > 0, 1.0 / np.maximum(cnt, 1), 0.0).astype(np.float32)
    rcp_t = np.zeros((128, 4), dtype=np.float32)
    rcp_t[:, :] = rcp.reshape(4, 128).T

    fcW = np.asarray(fcW, dtype=np.float32)
    consts = dict(
        iota_f=np.tile(np.arange(128, dtype=np.float32)[None, :], (128, 1)).astype(bf16),
        fcWm=fcW[:128].astype(bf16),
        fcWx=fcW[128:].astype(bf16),
        fcb=np.tile(np.asarray(fcb, dtype=np.float32)[None, :], (128, 1)),
        rcp_t=rcp_t,
    )
    return dict(G=G, T=T, layers=layers, idx_src_w=idx_src_w,
                idx_dst_w=idx_dst_w, O2_w=O2_w, x0T=x0T,
                segs=segs, consts=consts)


# ----------------------------------------------------------------------------
# Device program
# ----------------------------------------------------------------------------

def _build_program(p):
    import concourse.bass as bass
    import concourse.mybir as mybir
    import concourse.tile as tile
    from concourse import bacc

    G, T = p['G'], p['T']
    layers = p['layers']
    segs = p['segs']
    f32 = mybir.dt.float32
    bf = mybir.dt.bfloat16
    AX = mybir.AxisListType.X
    OP = mybir.AluOpType
    ACT = mybir.ActivationFunctionType

    nc = bacc.Bacc("TRN2", target_bir_lowering=False, debug=False,
                   num_devices=NCORES, num_swdge_queues=4)

    GI = G // 16              # idx cols per (w,cc) group
    G128 = G // 128           # gather out tiles per chunk

    # ---- I/O ----
    t_x0T = nc.dram_tensor("x0T", [32, NODES_PC], bf, kind="ExternalInput")
    t_isrc = nc.dram_tensor("isrc", [128, WPC * NCHUNK * GI], mybir.dt.int16,
                            kind="ExternalInput")
    t_idst = nc.dram_tensor("idst", [128, WPC * NCHUNK * GI], mybir.dt.int16,
                            kind="ExternalInput")
    t_drel = nc.dram_tensor("drel", [128, WPC * T], bf, kind="ExternalInput")
    t_iota = nc.dram_tensor("iota", [128, 128], bf, kind="ExternalInput")
    t_W, t_WAd, t_WAs, t_bias = [], [], [], []
    for li, L in enumerate(layers):
        t_W.append(nc.dram_tensor(f"W{li}", [L['cin'], 128], bf, kind="ExternalInput"))
        t_WAd.append(nc.dram_tensor(f"WAd{li}", [L['cin'], 8], bf, kind="ExternalInput"))
        t_WAs.append(nc.dram_tensor(f"WAs{li}", [L['cin'], 8], bf, kind="ExternalInput"))
        t_bias.append(nc.dram_tensor(f"bias{li}", [128, 128], f32, kind="ExternalInput"))
    t_fcWm = nc.dram_tensor("fcWm", [128, 6], bf, kind="ExternalInput")
    t_fcWx = nc.dram_tensor("fcWx", [128, 6], bf, kind="ExternalInput")
    t_fcb = nc.dram_tensor("fcb", [128, 6], f32, kind="ExternalInput")
    t_rcp = nc.dram_tensor("rcp", [128, 4], f32, kind="ExternalInput")
    t_out = nc.dram_tensor("out", [N_GRAPHS, 6], f32, kind="ExternalOutput")

    with tile.TileContext(nc) as tc:
        with (
            tc.tile_pool(name="const", bufs=1) as cpool,
            tc.tile_pool(name="persist", bufs=1) as ppool,
            tc.tile_pool(name="work", bufs=3) as wpool,
            tc.tile_pool(name="gath", bufs=2) as gpool,
            tc.tile_pool(name="psum", bufs=2, space="PSUM") as pspool,
            tc.tile_pool(name="psw", bufs=2, space="PSUM") as pswin,
            tc.tile_pool(name="dram", bufs=1, space="DRAM") as dpool,
        ):
            # ---- load constants ----
            def cload(t, shape, dtyp):
                s = cpool.tile(shape, dtyp, tag=t.name)
                nc.sync.dma_start(out=s[:], in_=t[:, :])
                return s
            fcWm = cload(t_fcWm, [128, 6], bf)
            fcWx = cload(t_fcWx, [128, 6], bf)
            fcb = cload(t_fcb, [128, 6], f32)
            rcp_t = cload(t_rcp, [128, 4], f32)
            Wsb, WAdsb, WAssb, biases = [], [], [], []
            for li, L in enumerate(layers):
                Wsb.append(cload(t_W[li], [L['cin'], 128], bf))
                WAdsb.append(cload(t_WAd[li], [L['cin'], 8], bf))
                WAssb.append(cload(t_WAs[li], [L['cin'], 8], bf))
                biases.append(cload(t_bias[li], [128, 128], f32))
            isrc_sb = ppool.tile([128, WPC * NCHUNK * GI], mybir.dt.int16)
            nc.sync.dma_start(out=isrc_sb[:], in_=t_isrc[:, :])
            idst_sb = ppool.tile([128, WPC * NCHUNK * GI], mybir.dt.int16)
            nc.sync.dma_start(out=idst_sb[:], in_=t_idst[:, :])
            x0T_sb = ppool.tile([32, NODES_PC], bf)
            nc.sync.dma_start(out=x0T_sb[:], in_=t_x0T[:, :])

            zero64 = cpool.tile([128, 64], bf, tag="zero64")
            nc.vector.memset(zero64[:], 0.0)

            # gather landing buffers (manually double-buffered): memset once so
            # skipped (padding) slots hold finite stale data instead of garbage
            hg_bufs = [ppool.tile([128, T, ROW], bf, name=f"hgbuf{i}")
                       for i in range(2)]
            ad_bufs = [ppool.tile([128, T, 128], bf, name=f"adbuf{i}")
                       for i in range(2)]
            for b_ in hg_bufs + ad_bufs:
                nc.vector.memset(b_[:], 0.0)

            # ---- build one-hot O2 once on-device (layer-independent) ----
            iota_f = cload(t_iota, [128, 128], bf)
            drel_sb = ppool.tile([128, WPC * T], bf)
            nc.sync.dma_start(out=drel_sb[:], in_=t_drel[:, :])
            O2d = dpool.tile([WPC * 128, T * 128], bf, tag="O2d")
            for w in range(WPC):
                O2b = wpool.tile([128, T * 128], bf, tag="O2b")
                for t in range(T):
                    nc.vector.tensor_tensor(
                        O2b[:, t * 128:(t + 1) * 128],
                        drel_sb[:, w * T + t:w * T + t + 1].to_broadcast([128, 128]),
                        iota_f[:], op=OP.is_equal)
                nc.sync.dma_start(out=O2d[w * 128:(w + 1) * 128, :], in_=O2b[:])

            out_own = None  # [NODES_PC, 128] DRAM, own shard of prev layer out
            for li, L in enumerate(layers):
                cin, c, hout = L['cin'], L['c'], L['hout']
                # ---- node phase: h | als rows + ad table for own shard ----
                h_shard = dpool.tile([NODES_PC, HROW], bf, tag="h_shard")
                att_shard = dpool.tile([NODES_PC, 128], bf, tag="att_shard")
                h_tbl_c = dpool.tile([NPAD, HROW], bf, tag="h_tbl_c")
                h_tbl = dpool.tile([NPAD, ROW], bf, tag="h_tbl")
                for nt in range(WPC):
                    if li == 0:
                        xT = x0T_sb[:cin, nt * 128:(nt + 1) * 128]
                    else:
                        xTt = wpool.tile([cin, 128], bf, tag="xT")
                        nc.sync.dma_start(
                            out=xTt[:],
                            in_=out_own[nt * 128:(nt + 1) * 128, :],
                            transpose=True)
                        xT = xTt[:]
                    nd_ps = pspool.tile([128, 144], f32, tag="nd_ps", space="PSUM")
                    nc.tensor.matmul(nd_ps[:, :128], xT, Wsb[li][:], start=True, stop=True)
                    nc.tensor.matmul(nd_ps[:, 128:136], xT, WAssb[li][:], start=True, stop=True)
                    nc.tensor.matmul(nd_ps[:, 136:144], xT, WAdsb[li][:], start=True, stop=True)
                    h_row = wpool.tile([128, HROW], bf, tag="h_row")
                    nc.vector.tensor_copy(h_row[:], nd_ps[:, :HROW])
                    ad_row = wpool.tile([128, 8], bf, tag="ad_row")
                    nc.vector.tensor_copy(ad_row[:], nd_ps[:, 136:144])
                    nc.sync.dma_start(out=h_shard[nt * 128:(nt + 1) * 128, :], in_=h_row[:])
                    nc.sync.dma_start(out=att_shard[nt * 128:(nt + 1) * 128, 0:8],
                                      in_=ad_row[:])
                # AllGather compact rows, then expand to 512B-strided table
                nc.gpsimd.collective_compute(
                    "AllGather", OP.bypass,
                    replica_groups=[list(range(NCORES))],
                    ins=[h_shard.opt()], outs=[h_tbl_c.opt()])
                for xc in range(4):
                    r0, r1 = xc * (NPAD // 4), (xc + 1) * (NPAD // 4)
                    nc.sync.dma_start(out=h_tbl[r0:r1, 0:HROW], in_=h_tbl_c[r0:r1, :])

                # ---- edge phase ----
                no_dstg = bool(int(os.environ.get("GAT_NO_DSTG", "0")))
                no_o2 = bool(int(os.environ.get("GAT_NO_O2", "0")))
                out_new = dpool.tile([NODES_PC, 128], bf, tag="out_own")
                for w in range(WPC):
                    hg_win = hg_bufs[w % 2]
                    ad_win = ad_bufs[w % 2]
                    for cc in range(NCHUNK):
                        nc.gpsimd.dma_gather(
                            out_ap=hg_win[:, cc * G128:(cc + 1) * G128, :],
                            in_ap=h_tbl[cc * CHUNK:(cc + 1) * CHUNK, :],
                            idxs_ap=isrc_sb[:, (w * NCHUNK + cc) * GI:
                                            (w * NCHUNK + cc + 1) * GI],
                            num_idxs=G, num_idxs_reg=G, elem_size=ROW,
                            single_packet=False, queue_num=cc)
                    if not no_dstg:
                        nc.gpsimd.dma_gather(
                            out_ap=ad_win[:, :, :],
                            in_ap=att_shard[:, :],
                            idxs_ap=idst_sb[:, w * NCHUNK * GI:(w + 1) * NCHUNK * GI],
                            num_idxs=NCHUNK * G, num_idxs_reg=NCHUNK * G, elem_size=128,
                            single_packet=False, queue_num=w % 4)
                    O2w = gpool.tile([128, T * 128], bf, tag="O2w")
                    if no_o2:
                        nc.vector.memset(O2w[:], 0.0)
                    else:
                        nc.sync.dma_start(out=O2w[:],
                                          in_=O2d[w * 128:(w + 1) * 128, :])
                    ps_win = pswin.tile([128, 136], f32, tag="ps_win", space="PSUM")
                    # whole-window z pipeline (batched: one op per stage)
                    zall = wpool.tile([128, T, 8], f32, tag="z")
                    nc.vector.tensor_add(zall[:], hg_win[:, :, 128:136],
                                         ad_win[:, :, 0:8])
                    zl = wpool.tile([128, T, 8], f32, tag="zl")
                    nc.vector.scalar_tensor_tensor(zl[:], zall[:], 0.2, zall[:],
                                                   op0=OP.mult, op1=OP.max)
                    msg_all = wpool.tile([128, T, 136], bf, tag="msg")
                    nc.scalar.activation(msg_all[:, :, 128:136], zl[:], ACT.Exp)
                    nc.vector.tensor_mul(
                        msg_all[:, :, :hout].rearrange("p t (h c) -> p t h c", c=c),
                        hg_win[:, :, :hout].rearrange("p t (h c) -> p t h c", c=c),
                        msg_all[:, :, 128:136].rearrange("p t (h c) -> p t h c", c=1)
                        .to_broadcast([128, T, 8, c]))
                    for t in range(T):
                        first = (t == 0)
                        last = (t == T - 1)
                        O2t = O2w[:, t * 128:(t + 1) * 128]
                        if hout == 128:
                            nc.tensor.matmul(ps_win[:, :136], O2t,
                                             msg_all[:, t, :],
                                             start=first, stop=last)
                        else:
                            nc.tensor.matmul(ps_win[:, :hout], O2t,
                                             msg_all[:, t, :hout],
                                             start=first, stop=last)
                            nc.tensor.matmul(ps_win[:, 128:136], O2t,
                                             msg_all[:, t, 128:136],
                                             start=first, stop=last)
                    # ---- flush window ----
                    den = wpool.tile([128, 8], f32, tag="den")
                    nc.vector.tensor_scalar_max(den[:], ps_win[:, 128:136], 1e-20)
                    rcpd = wpool.tile([128, 8], f32, tag="rcpd")
                    nc.vector.reciprocal(rcpd[:], den[:])
                    o1 = wpool.tile([128, hout], f32, tag="o1")
                    nc.vector.tensor_mul(
                        o1[:].rearrange("p (h c) -> p h c", c=c),
                        ps_win[:, :hout].rearrange("p (h c) -> p h c", c=c),
                        rcpd[:].rearrange("p (h o) -> p h o", o=1).to_broadcast([128, 8, c]))
                    if L['has_bias']:
                        nc.vector.tensor_add(o1[:], o1[:], biases[li][:, :hout])
                    # ELU on ScalarE: relu(x) + exp(-relu(-x)) - 1
                    pos = wpool.tile([128, hout], bf, tag="pos")
                    nc.scalar.activation(pos[:], o1[:], ACT.Relu)
                    nneg = wpool.tile([128, hout], f32, tag="nneg")
                    nc.scalar.activation(nneg[:], o1[:], ACT.Relu, scale=-1.0)
                    en = wpool.tile([128, hout], f32, tag="en")
                    nc.scalar.activation(en[:], nneg[:], ACT.Exp, scale=-1.0)
                    ob = wpool.tile([128, hout], bf, tag="ob")
                    nc.vector.scalar_tensor_tensor(ob[:], en[:], -1.0, pos[:],
                                                   op0=OP.add, op1=OP.add)
                    nc.sync.dma_start(out=out_new[w * 128:(w + 1) * 128, :hout], in_=ob[:])
                    if hout < 128:
                        nc.sync.dma_start(out=out_new[w * 128:(w + 1) * 128, hout:],
                                          in_=zero64[:])
                out_own = out_new

            # ---- final AllGather of out4 + pooling ----
            out4 = dpool.tile([NPAD, 128], bf, tag="out4")
            nc.gpsimd.collective_compute(
                "AllGather", OP.bypass,
                replica_groups=[list(range(NCORES))],
                ins=[out_own.opt()], outs=[out4.opt()])

            sumsT = ppool.tile([128, N_GRAPHS], f32)
            mxT = ppool.tile([128, N_GRAPHS], f32)
            nc.vector.memset(sumsT[:], 0.0)
            nc.vector.memset(mxT[:], 0.0)
            blk_tiles = {}
            needed_blocks = sorted({s[0] for gs in segs for s in gs})
            for blk in needed_blocks:
                bt = wpool.tile([128, POOL_BLOCK], bf, tag="poolblk")
                nc.sync.dma_start(out=bt[:],
                                  in_=out4[blk * POOL_BLOCK:(blk + 1) * POOL_BLOCK, :],
                                  transpose=True)
                blk_tiles[blk] = bt
                for g, gs in enumerate(segs):
                    if not gs or gs[-1][0] != blk:
                        continue
                    if len(gs) == 1:
                        b0, c0, wd = gs[0]
                        nc.vector.reduce_sum(sumsT[:, g:g + 1],
                                             blk_tiles[b0][:, c0:c0 + wd], axis=AX)
                        nc.vector.reduce_max(mxT[:, g:g + 1],
                                             blk_tiles[b0][:, c0:c0 + wd], axis=AX)
                    else:
                        tmp_s = wpool.tile([128, 1], f32, tag="tmp_s")
                        tmp_m = wpool.tile([128, 1], f32, tag="tmp_m")
                        first = True
                        for (b0, c0, wd) in gs:
                            if first:
                                nc.vector.reduce_sum(sumsT[:, g:g + 1],
                                                     blk_tiles[b0][:, c0:c0 + wd], axis=AX)
                                nc.vector.reduce_max(mxT[:, g:g + 1],
                                                     blk_tiles[b0][:, c0:c0 + wd], axis=AX)
                                first = False
                            else:
                                nc.vector.reduce_sum(tmp_s[:],
                                                     blk_tiles[b0][:, c0:c0 + wd], axis=AX)
                                nc.vector.reduce_max(tmp_m[:],
                                                     blk_tiles[b0][:, c0:c0 + wd], axis=AX)
                                nc.vector.tensor_add(sumsT[:, g:g + 1],
                                                     sumsT[:, g:g + 1], tmp_s[:])
                                nc.vector.tensor_max(mxT[:, g:g + 1],
                                                     mxT[:, g:g + 1], tmp_m[:])
            sumsTb = ppool.tile([128, N_GRAPHS], bf)
            mxTb = ppool.tile([128, N_GRAPHS], bf)
            nc.vector.tensor_copy(sumsTb[:], sumsT[:])
            nc.vector.tensor_copy(mxTb[:], mxT[:])

            for gt in range(N_GRAPHS // 128):
                p1 = pspool.tile([128, 6], f32, tag="p1", space="PSUM")
                p2 = pspool.tile([128, 6], f32, tag="p2", space="PSUM")
                nc.tensor.matmul(p1[:], sumsTb[:, gt * 128:(gt + 1) * 128], fcWm[:],
                                 start=True, stop=True)
                nc.tensor.matmul(p2[:], mxTb[:, gt * 128:(gt + 1) * 128], fcWx[:],
                                 start=True, stop=True)
                zz = wpool.tile([128, 6], f32, tag="zz")
                nc.vector.tensor_mul(zz[:], p1[:],
                                     rcp_t[:, gt:gt + 1].to_broadcast([128, 6]))
                nc.vector.tensor_add(zz[:], zz[:], p2[:])
                nc.vector.tensor_add(zz[:], zz[:], fcb[:])
                m6 = wpool.tile([128, 1], f32, tag="m6")
                nc.vector.reduce_max(m6[:], zz[:], axis=AX)
                nc.vector.tensor_sub(zz[:], zz[:], m6[:].to_broadcast([128, 6]))
                e6 = wpool.tile([128, 6], f32, tag="e6")
                nc.scalar.activation(e6[:], zz[:], ACT.Exp)
                s6 = wpool.tile([128, 1], f32, tag="s6")
                nc.vector.reduce_sum(s6[:], e6[:], axis=AX)
                nc.scalar.activation(s6[:], s6[:], ACT.Ln)
                nc.vector.tensor_sub(zz[:], zz[:], s6[:].to_broadcast([128, 6]))
                nc.sync.dma_start(out=t_out[gt * 128:(gt + 1) * 128, :], in_=zz[:])
    nc.compile()
    return nc


def _kernel_trn(x, edge_index, batch,
                W1, a1s, a1d, b1, W2, a2s, a2d, b2,
                W3, a3s, a3d, b3, W4, a4s, a4d, b4, fcW, fcb):
    global _LAST_EXEC_NS
    from concourse.bass_utils import run_bass_kernel_spmd

    p = _prep(np.asarray(x), np.asarray(edge_index), np.asarray(batch),
              [W1, W2, W3, W4], [a1d, a2d, a3d, a4d], [a1s, a2s, a3s, a4s],
              [b1, b2, b3, b4], fcW, fcb)
    nc = _build_program(p)

    in_maps = []
    for cidx in range(NCORES):
        m = dict(
            x0T=p['x0T'][cidx], isrc=p['idx_src_w'][cidx],
            idst=p['idx_dst_w'][cidx], drel=p['O2_w'][cidx],
            iota=p['consts']['iota_f'],
            fcWm=p['consts']['fcWm'], fcWx=p['consts']['fcWx'],
            fcb=p['consts']['fcb'], rcp=p['consts']['rcp_t'],
        )
        for li, L in enumerate(p['layers']):
            m[f"W{li}"] = L['W']
            m[f"WAd{li}"] = L['WAd']
            m[f"WAs{li}"] = L['WAs']
            m[f"bias{li}"] = L['bias']
        in_maps.append(m)

    trace = bool(int(os.environ.get("GAT_TRACE", "0")))
    tdir = os.environ.get("GAT_TRACE_DIR") or None
    res = run_bass_kernel_spmd(nc, in_maps, core_ids=list(range(NCORES)),
                               trace=trace, tmpdir=tdir)
    _LAST_EXEC_NS = res.exec_time_ns
    globals()['_LAST_RES'] = res
    return np.asarray(res.results[0]["out"], dtype=np.float32)


# ----------------------------------------------------------------------------
# numpy fallback (previous baseline)
# ----------------------------------------------------------------------------

def _leaky_relu(v, slope=0.2):
    return np.where(v >= 0, v, slope * v)


def _elu(v):
    return np.where(v > 0, v, np.expm1(np.minimum(v, 0.0)))


def _gat_layer_np(x, W, a_src, a_dst, b, src_s, dst_s, starts, n, c):
    h = (x @ W).reshape(n, H, c)
    al_s = np.einsum('nhc,hc->nh', h, a_src)
    al_d = np.einsum('nhc,hc->nh', h, a_dst)
    e = _leaky_relu(al_s[src_s] + al_d[dst_s])
    e_max = np.maximum.reduceat(e, starts, axis=0)
    ex = np.exp(e - e_max[dst_s])
    denom = np.add.reduceat(ex, starts, axis=0)
    alpha = ex / denom[dst_s]
    msg = h[src_s]
    msg *= alpha[:, :, None]
    out = np.add.reduceat(msg.reshape(-1, H * c), starts, axis=0)
    return out + b


def _kernel_numpy(x, edge_index, batch,
                  W1, a1s, a1d, b1, W2, a2s, a2d, b2,
                  W3, a3s, a3d, b3, W4, a4s, a4d, b4, fcW, fcb):
    x = np.asarray(x, dtype=np.float32)
    edge_index = np.asarray(edge_index)
    batch = np.asarray(batch)
    n = x.shape[0]
    loops = np.arange(n, dtype=edge_index.dtype)
    src = np.concatenate([edge_index[0], loops])
    dst = np.concatenate([edge_index[1], loops])
    order = np.argsort(dst, kind='stable')
    src_s = src[order]
    dst_s = dst[order]
    counts = np.bincount(dst, minlength=n)
    starts = np.zeros(n, dtype=np.int64)
    np.cumsum(counts[:-1], out=starts[1:])
    x = _elu(_gat_layer_np(x, W1, a1s, a1d, b1, src_s, dst_s, starts, n, 8))
    x = _elu(_gat_layer_np(x, W2, a2s, a2d, b2, src_s, dst_s, starts, n, 16))
    x = _elu(_gat_layer_np(x, W3, a3s, a3d, b3, src_s, dst_s, starts, n, 16))
    x = _elu(_gat_layer_np(x, W4, a4s, a4d, b4, src_s, dst_s, starts, n, 16))
    cnt = np.bincount(batch, minlength=N_GRAPHS)
    nz = cnt > 0
    bstarts = np.zeros(N_GRAPHS, dtype=np.int64)
    np.cumsum(cnt[:-1], out=bstarts[1:])
    f = x.shape[1]
    mean = np.zeros((N_GRAPHS, f), dtype=np.float32)
    mx = np.zeros((N_GRAPHS, f), dtype=np.float32)
    nz_starts = bstarts[nz]
    mean[nz] = np.add.reduceat(x, nz_starts, axis=0) / cnt[nz, None]
    mx[nz] = np.maximum.reduceat(x, nz_starts, axis=0)
    feat = np.concatenate([mean, mx], axis=1)
    z = feat @ fcW + fcb
    z -= z.max(axis=1, keepdims=True)
    z -= np.log(np.exp(z).sum(axis=1, keepdims=True))
    return z.astype(np.float32)


def kernel(**inputs):
    if not int(os.environ.get("GAT_FORCE_NUMPY", "0")):
        try:
            return _kernel_trn(**inputs)
        except Exception:
            import traceback
            traceback.print_exc()
    return _kernel_numpy(**inputs)


# revision 12
# speedup vs baseline: 8658.3730x; 7608.1044x over previous
"""GatNet on 8 Trainium2 NeuronCores (Bass/Tile).

4-layer GAT (8 heads) + mean/max graph pooling + FC + log_softmax.

Distribution: edges partitioned by destination node across the 8 cores
(each core owns a contiguous range of 12544 destination nodes and all edges
into them). Per layer:
  - node phase (sharded): h = x @ W and ad = x @ (W @ a_dst) for own nodes,
    AllGather of the h table (gather source for the edge phase).
  - edge phase: per 128-dst-node window, the incoming edges' h[src] rows are
    fetched with dma_gather (4 SWDGE queues, one per 25088-row chunk of the
    node table so indices fit int16), attention logits are built on-chip,
    and segment softmax numerator/denominator are accumulated with
    one-hot-indicator matmuls on TensorE (edges sorted by dst => each window
    is a fixed set of edge tiles; padding slots carry d_rel=128 so their
    one-hot column is all-zero and they drop out exactly).
  - flush: out = num/denom + b, ELU, store.
Pooling/FC run replicated on every core from the AllGathered final layer.

Self-contained: accepts the FULL inputs, returns the FULL [512, 6] output.
Falls back to a pure-numpy implementation if the device path fails.
"""
import os
import numpy as np

H = 8
N_GRAPHS = 512
NPAD = 100352            # 784 * 128
NCORES = 8
NCHUNK = 4
CHUNK = NPAD // NCHUNK   # 25088
WPC = NPAD // 128 // NCORES   # 98 windows (node tiles) per core
NODES_PC = NPAD // NCORES     # 12544
POOL_BLOCK = 1024

_LAST_EXEC_NS = None     # set by the device path; read by test harness


# ----------------------------------------------------------------------------
# CPU prep
# ----------------------------------------------------------------------------

def _build_edge_streams(src, dst):
    """Group edges (dst-sorted) by (128-dst-window, src-chunk); pad each group
    to a common G slots. Returns idx_slots [784,4,G] int16 (chunk-local src),
    drel_slots [784,4,G] float32 (dst - window_base; 128 for padding)."""
    order = np.argsort(dst, kind='stable')
    src_s = src[order].astype(np.int64)
    dst_s = dst[order].astype(np.int64)
    win = dst_s >> 7
    chunk = src_s // CHUNK
    order2 = np.lexsort((chunk, win))
    src_s = src_s[order2]
    dst_s = dst_s[order2]
    win = win[order2]
    chunk = chunk[order2]
    gidx = win * NCHUNK + chunk
    counts = np.bincount(gidx, minlength=784 * NCHUNK)
    G = int(counts.max())
    G = -(-G // 128) * 128                     # output slots need 128-alignment
    starts = np.zeros(784 * NCHUNK, dtype=np.int64)
    np.cumsum(counts[:-1], out=starts[1:])
    # slot arrays via vectorized scatter
    n_e = src_s.shape[0]
    slot_of_edge = np.repeat(starts, counts)   # group start per edge (grouped order)
    within = np.arange(n_e) - slot_of_edge
    flat_slot = gidx * G + within
    idx_slots = np.zeros(784 * NCHUNK * G, dtype=np.int16)
    drel_slots = np.full(784 * NCHUNK * G, 128.0, dtype=np.float32)
    idx_slots[flat_slot] = (src_s - chunk * CHUNK).astype(np.int16)
    drel_slots[flat_slot] = (dst_s - (win << 7)).astype(np.float32)
    return idx_slots.reshape(784, NCHUNK, G), drel_slots.reshape(784, NCHUNK, G), G


def _graph_segments(batch):
    """Per graph: list of (block, col0, width) segments within POOL_BLOCK node
    blocks, plus counts."""
    cnt = np.bincount(batch, minlength=N_GRAPHS).astype(np.int64)
    bstarts = np.zeros(N_GRAPHS, dtype=np.int64)
    np.cumsum(cnt[:-1], out=bstarts[1:])
    segs = []
    for g in range(N_GRAPHS):
        s, e = int(bstarts[g]), int(bstarts[g] + cnt[g])
        out = []
        while s < e:
            blk = s // POOL_BLOCK
            e_blk = min(e, (blk + 1) * POOL_BLOCK)
            out.append((blk, s - blk * POOL_BLOCK, e_blk - s))
            s = e_blk
        segs.append(out)
    return segs, cnt


def _prep(x, edge_index, batch, Ws, Ads, Ass, bs, fcW, fcb):
    """All CPU-side preparation. Returns dict of per-core inputs + constants."""
    import ml_dtypes
    bf16 = ml_dtypes.bfloat16

    n = x.shape[0]
    loops = np.arange(n, dtype=np.int64)
    src = np.concatenate([np.asarray(edge_index[0], dtype=np.int64), loops])
    dst = np.concatenate([np.asarray(edge_index[1], dtype=np.int64), loops])
    idx_slots, drel_slots, G = _build_edge_streams(src, dst)
    T = NCHUNK * G // 128

    # per-core idx stream wrapped [16, .] then replicated to 128 partitions
    idx_w = []
    drel_w = []
    for cidx in range(NCORES):
        a = idx_slots[cidx * WPC:(cidx + 1) * WPC]          # [98, 4, G]
        aw = a.reshape(WPC * NCHUNK, G // 16, 16)
        aw = np.moveaxis(aw, 2, 0).reshape(16, WPC * NCHUNK * (G // 16))
        idx_w.append(np.tile(np.ascontiguousarray(aw), (8, 1)))
        d = drel_slots[cidx * WPC:(cidx + 1) * WPC]         # [98, 4, G]
        dv = d.reshape(WPC, T, 128)
        dv = np.moveaxis(dv, 2, 0).reshape(128, WPC * T)
        drel_w.append(np.ascontiguousarray(dv).astype(bf16))

    # layer dims: (cin_pad, c, hout)
    dims = [(32, 8, 64), (128, 16, 128), (128, 16, 128), (128, 16, 128)]
    layers = []
    for li, (cin, c, hout) in enumerate(dims):
        Wp = np.zeros((cin, 128), dtype=np.float32)
        Wr = np.asarray(Ws[li], dtype=np.float32)
        Wp[:Wr.shape[0], :Wr.shape[1]] = Wr
        Ad = np.asarray(Ads[li], dtype=np.float32)
        As = np.asarray(Ass[li], dtype=np.float32)
        WAd = np.zeros((cin, 8), dtype=np.float32)
        for h in range(H):
            WAd[:, h] = Wp[:, h * c:(h + 1) * c] @ Ad[h]
        As_t = np.zeros(128, dtype=np.float32)
        As_t[:H * c] = As.reshape(-1)
        b = np.zeros(128, dtype=np.float32)
        b[:hout] = np.asarray(bs[li], dtype=np.float32)
        layers.append(dict(
            cin=cin, c=c, hout=hout,
            W=Wp.astype(bf16),
            WAd=WAd.astype(bf16),
            As_b=np.tile(As_t[None, :], (128, 1)).astype(bf16),
            bias=np.tile(b[None, :], (128, 1)).astype(np.float32),
            has_bias=bool(np.any(b != 0)),
        ))

    # x shards, transposed [32, 12544] bf16
    xp = np.zeros((NPAD, 32), dtype=np.float32)
    xp[:n, :x.shape[1]] = np.asarray(x, dtype=np.float32)
    x0T = [np.ascontiguousarray(xp[cidx * NODES_PC:(cidx + 1) * NODES_PC].T).astype(bf16)
           for cidx in range(NCORES)]

    segs, cnt = _graph_segments(np.asarray(batch, dtype=np.int64))
    rcp = np.where(cnt > 0, 1.0 / np.maximum(cnt, 1), 0.0).astype(np.float32)
    rcp_t = np.zeros((128, 4), dtype=np.float32)
    rcp_t[:, :] = rcp.reshape(4, 128).T

    fcW = np.asarray(fcW, dtype=np.float32)
    consts = dict(
        iota_f=np.tile(np.arange(128, dtype=np.float32)[None, :], (128, 1)).astype(bf16),
        iota_p=np.tile(np.arange(128, dtype=np.float32)[:, None], (1, 128)).astype(bf16),
        ident=np.eye(128, dtype=np.float32).astype(bf16),
        fcWm=fcW[:128].astype(bf16),
        fcWx=fcW[128:].astype(bf16),
        fcb=np.tile(np.asarray(fcb, dtype=np.float32)[None, :], (128, 1)),
        rcp_t=rcp_t,
    )
    return dict(G=G, T=T, layers=layers, idx_w=idx_w, drel_w=drel_w, x0T=x0T,
                segs=segs, consts=consts)


# ----------------------------------------------------------------------------
# Device program
# ----------------------------------------------------------------------------

def _build_program(p):
    import concourse.bass as bass
    import concourse.mybir as mybir
    import concourse.tile as tile
    from concourse import bacc

    G, T = p['G'], p['T']
    layers = p['layers']
    segs = p['segs']
    f32 = mybir.dt.float32
    bf = mybir.dt.bfloat16
    AX = mybir.AxisListType.X
    OP = mybir.AluOpType
    ACT = mybir.ActivationFunctionType

    nc = bacc.Bacc("TRN2", target_bir_lowering=False, debug=False,
                   num_devices=NCORES, num_swdge_queues=4)

    # ---- I/O ----
    t_x0T = nc.dram_tensor("x0T", [32, NODES_PC], bf, kind="ExternalInput")
    t_idx = nc.dram_tensor("idx", [128, WPC * NCHUNK * G // 16], mybir.dt.int16,
                           kind="ExternalInput")
    t_drel = nc.dram_tensor("drel", [128, WPC * T], bf, kind="ExternalInput")
    t_W, t_WAd, t_Asb, t_bias = [], [], [], []
    for li, L in enumerate(layers):
        t_W.append(nc.dram_tensor(f"W{li}", [L['cin'], 128], bf, kind="ExternalInput"))
        t_WAd.append(nc.dram_tensor(f"WAd{li}", [L['cin'], 8], bf, kind="ExternalInput"))
        t_Asb.append(nc.dram_tensor(f"Asb{li}", [128, 128], bf, kind="ExternalInput"))
        t_bias.append(nc.dram_tensor(f"bias{li}", [128, 128], f32, kind="ExternalInput"))
    t_iota_f = nc.dram_tensor("iota_f", [128, 128], bf, kind="ExternalInput")
    t_iota_p = nc.dram_tensor("iota_p", [128, 128], bf, kind="ExternalInput")
    t_ident = nc.dram_tensor("ident", [128, 128], bf, kind="ExternalInput")
    t_fcWm = nc.dram_tensor("fcWm", [128, 6], bf, kind="ExternalInput")
    t_fcWx = nc.dram_tensor("fcWx", [128, 6], bf, kind="ExternalInput")
    t_fcb = nc.dram_tensor("fcb", [128, 6], f32, kind="ExternalInput")
    t_rcp = nc.dram_tensor("rcp", [128, 4], f32, kind="ExternalInput")
    t_out = nc.dram_tensor("out", [N_GRAPHS, 6], f32, kind="ExternalOutput")

    with tile.TileContext(nc) as tc:
        with (
            tc.tile_pool(name="const", bufs=1) as cpool,
            tc.tile_pool(name="persist", bufs=1) as ppool,
            tc.tile_pool(name="work", bufs=3) as wpool,
            tc.tile_pool(name="gath", bufs=2) as gpool,
            tc.tile_pool(name="psum", bufs=2, space="PSUM") as pspool,
            tc.tile_pool(name="psw", bufs=2, space="PSUM") as pswin,
            tc.tile_pool(name="dram", bufs=1, space="DRAM") as dpool,
        ):
            # ---- load constants ----
            def cload(t, shape, dtyp):
                s = cpool.tile(shape, dtyp, tag=t.name)
                nc.sync.dma_start(out=s[:], in_=t[:, :])
                return s
            iota_f = cload(t_iota_f, [128, 128], bf)
            iota_p = cload(t_iota_p, [128, 128], bf)
            ident = cload(t_ident, [128, 128], bf)
            fcWm = cload(t_fcWm, [128, 6], bf)
            fcWx = cload(t_fcWx, [128, 6], bf)
            fcb = cload(t_fcb, [128, 6], f32)
            rcp_t = cload(t_rcp, [128, 4], f32)
            Wsb, WAdsb, Asbs, biases = [], [], [], []
            for li, L in enumerate(layers):
                Wsb.append(cload(t_W[li], [L['cin'], 128], bf))
                WAdsb.append(cload(t_WAd[li], [L['cin'], 8], bf))
                Asbs.append(cload(t_Asb[li], [128, 128], bf))
                biases.append(cload(t_bias[li], [128, 128], f32))
            idx_sb = ppool.tile([128, WPC * NCHUNK * G // 16], mybir.dt.int16)
            nc.sync.dma_start(out=idx_sb[:], in_=t_idx[:, :])
            drel_sb = ppool.tile([128, WPC * T], bf)
            nc.sync.dma_start(out=drel_sb[:], in_=t_drel[:, :])
            x0T_sb = ppool.tile([32, NODES_PC], bf)
            nc.sync.dma_start(out=x0T_sb[:], in_=t_x0T[:, :])

            zero64 = cpool.tile([128, 64], bf, tag="zero64")
            nc.vector.memset(zero64[:], 0.0)

            ad_all = ppool.tile([128, WPC * 8], bf)  # own-shard a_dst logits per layer

            out_own = None  # [NODES_PC, hout] DRAM, own shard of previous layer out
            for li, L in enumerate(layers):
                cin, c, hout = L['cin'], L['c'], L['hout']
                # ---- node phase: h, ad for own shard ----
                h_shard = dpool.tile([NODES_PC, 128], bf, tag="h_shard")
                h_tbl = dpool.tile([NPAD, 128], bf, tag="h_tbl")
                nblk = NODES_PC // POOL_BLOCK  # 12.25 -> handle remainder
                # transposed x tiles: from x0T (layer 0) or DMA-transpose of out_own
                for nt in range(WPC):
                    if li == 0:
                        xT = x0T_sb[:cin, nt * 128:(nt + 1) * 128]
                    else:
                        xTt = wpool.tile([cin, 128], bf, tag="xT")
                        nc.sync.dma_start(
                            out=xTt[:],
                            in_=out_own[nt * 128:(nt + 1) * 128, :],
                            transpose=True)
                        xT = xTt[:]
                    nd_ps = pspool.tile([128, 136], f32, tag="nd_ps", space="PSUM")
                    nc.tensor.matmul(nd_ps[:, :128], xT, Wsb[li][:], start=True, stop=True)
                    nc.tensor.matmul(nd_ps[:, 128:136], xT, WAdsb[li][:], start=True, stop=True)
                    h_sb = wpool.tile([128, 128], bf, tag="h_sb")
                    nc.vector.tensor_copy(h_sb[:], nd_ps[:, :128])
                    nc.vector.tensor_copy(ad_all[:, nt * 8:(nt + 1) * 8], nd_ps[:, 128:136])
                    nc.sync.dma_start(out=h_shard[nt * 128:(nt + 1) * 128, :], in_=h_sb[:])
                # AllGather h
                nc.gpsimd.collective_compute(
                    "AllGather", OP.bypass,
                    replica_groups=[list(range(NCORES))],
                    ins=[h_shard.opt()], outs=[h_tbl.opt()])

                # ---- edge phase ----
                out_new = dpool.tile([NODES_PC, 128], bf, tag="out_own")
                for w in range(WPC):
                    hg_win = gpool.tile([128, T, 128], bf, tag="hg")
                    for cc in range(NCHUNK):
                        nc.gpsimd.dma_gather(
                            out_ap=hg_win[:, cc * (G // 128):(cc + 1) * (G // 128), :],
                            in_ap=h_tbl[cc * CHUNK:(cc + 1) * CHUNK, :],
                            idxs_ap=idx_sb[:, (w * NCHUNK + cc) * (G // 16):
                                           (w * NCHUNK + cc + 1) * (G // 16)],
                            num_idxs=G, num_idxs_reg=G, elem_size=128,
                            single_packet=False, queue_num=cc)
                    ps_win = pswin.tile([128, 136], f32, tag="ps_win", space="PSUM")
                    ad_w = ad_all[:, w * 8:(w + 1) * 8]
                    for tp in range(T // 2):
                        t0 = 2 * tp
                        hg2 = hg_win[:, t0:t0 + 2, :]               # [128, 2, 128]
                        d2 = drel_sb[:, w * T + t0:w * T + t0 + 2]  # [128, 2]
                        # als2[e, (o,h)] = sum_c hg * As
                        tmp2 = wpool.tile([128, 2, 128], f32, tag="tmp")
                        nc.vector.tensor_mul(
                            tmp2[:], hg2,
                            Asbs[li][:].rearrange("p (o f) -> p o f", o=1)
                            .to_broadcast([128, 2, 128]))
                        als2 = wpool.tile([128, 16], f32, tag="als")
                        nc.vector.tensor_reduce(
                            als2[:].rearrange("p (o h) -> p o h", o=2),
                            tmp2[:, :, :hout].rearrange("p o (h c) -> p o h c", c=c),
                            axis=AX, op=OP.add)
                        # one-hot O2 [e, (o,s)] and O_T2 [s, (o,e)]
                        dT2 = pspool.tile([128, 2, 128], bf, tag="dT", space="PSUM")
                        nc.tensor.transpose(dT2[:, 0, :],
                                            d2[:, 0:1].to_broadcast([128, 128]),
                                            ident[:])
                        nc.tensor.transpose(dT2[:, 1, :],
                                            d2[:, 1:2].to_broadcast([128, 128]),
                                            ident[:])
                        O_T2 = wpool.tile([128, 2, 128], bf, tag="OT")
                        nc.vector.tensor_tensor(
                            O_T2[:], dT2[:],
                            iota_p[:].rearrange("p (o f) -> p o f", o=1)
                            .to_broadcast([128, 2, 128]), op=OP.is_equal)
                        O2 = wpool.tile([128, 2, 128], bf, tag="O")
                        nc.vector.tensor_tensor(
                            O2[:],
                            d2[:].rearrange("p (o f) -> p o f", f=1)
                            .to_broadcast([128, 2, 128]),
                            iota_f[:].rearrange("p (o f) -> p o f", o=1)
                            .to_broadcast([128, 2, 128]), op=OP.is_equal)
                        # ad per edge
                        ad_e2 = pspool.tile([128, 16], f32, tag="ad_e", space="PSUM")
                        nc.tensor.matmul(ad_e2[:, 0:8], O_T2[:, 0, :], ad_w,
                                         start=True, stop=True)
                        nc.tensor.matmul(ad_e2[:, 8:16], O_T2[:, 1, :], ad_w,
                                         start=True, stop=True)
                        z2t = wpool.tile([128, 16], f32, tag="z")
                        nc.vector.tensor_add(z2t[:], als2[:], ad_e2[:])
                        # leaky relu on DVE: max(z, 0.2 z)
                        zl = wpool.tile([128, 16], f32, tag="zl")
                        nc.vector.scalar_tensor_tensor(zl[:], z2t[:], 0.2, z2t[:],
                                                       op0=OP.mult, op1=OP.max)
                        msg2 = wpool.tile([128, 2, 136], bf, tag="msg")
                        nc.scalar.activation(
                            msg2[:, :, 128:136],
                            zl[:].rearrange("p (o h) -> p o h", o=2), ACT.Exp)
                        nc.vector.tensor_mul(
                            msg2[:, :, :hout].rearrange("p o (h c) -> p o h c", c=c),
                            hg2[:, :, :hout].rearrange("p o (h c) -> p o h c", c=c),
                            msg2[:, :, 128:136].rearrange("p o (h c) -> p o h c", c=1)
                            .to_broadcast([128, 2, 8, c]))
                        for t in (0, 1):
                            first = (t0 + t == 0)
                            last = (t0 + t == T - 1)
                            if hout == 128:
                                nc.tensor.matmul(ps_win[:, :136], O2[:, t, :],
                                                 msg2[:, t, :],
                                                 start=first, stop=last)
                            else:
                                nc.tensor.matmul(ps_win[:, :hout], O2[:, t, :],
                                                 msg2[:, t, :hout],
                                                 start=first, stop=last)
                                nc.tensor.matmul(ps_win[:, 128:136], O2[:, t, :],
                                                 msg2[:, t, 128:136],
                                                 start=first, stop=last)
                    # ---- flush window ----
                    den = wpool.tile([128, 8], f32, tag="den")
                    nc.vector.tensor_scalar_max(den[:], ps_win[:, 128:136], 1e-20)
                    rcpd = wpool.tile([128, 8], f32, tag="rcpd")
                    nc.vector.reciprocal(rcpd[:], den[:])
                    o1 = wpool.tile([128, hout], f32, tag="o1")
                    nc.vector.tensor_mul(
                        o1[:].rearrange("p (h c) -> p h c", c=c),
                        ps_win[:, :hout].rearrange("p (h c) -> p h c", c=c),
                        rcpd[:].rearrange("p (h o) -> p h o", o=1).to_broadcast([128, 8, c]))
                    if L['has_bias']:
                        nc.vector.tensor_add(o1[:], o1[:], biases[li][:, :hout])
                    neg = wpool.tile([128, hout], f32, tag="neg")
                    nc.vector.tensor_scalar_min(neg[:], o1[:], 0.0)
                    en = wpool.tile([128, hout], f32, tag="en")
                    nc.scalar.activation(en[:], neg[:], ACT.Exp)
                    pos = wpool.tile([128, hout], bf, tag="pos")
                    nc.vector.tensor_scalar_max(pos[:], o1[:], 0.0)
                    ob = wpool.tile([128, hout], bf, tag="ob")
                    nc.vector.scalar_tensor_tensor(ob[:], en[:], -1.0, pos[:],
                                                   op0=OP.add, op1=OP.add)
                    nc.sync.dma_start(out=out_new[w * 128:(w + 1) * 128, :hout], in_=ob[:])
                    if hout < 128:
                        nc.sync.dma_start(out=out_new[w * 128:(w + 1) * 128, hout:],
                                          in_=zero64[:])
                out_own = out_new

            # ---- final AllGather of out4 + pooling ----
            out4 = dpool.tile([NPAD, 128], bf, tag="out4")
            nc.gpsimd.collective_compute(
                "AllGather", OP.bypass,
                replica_groups=[list(range(NCORES))],
                ins=[out_own.opt()], outs=[out4.opt()])

            sumsT = ppool.tile([128, N_GRAPHS], f32)
            mxT = ppool.tile([128, N_GRAPHS], f32)
            nc.vector.memset(sumsT[:], 0.0)
            nc.vector.memset(mxT[:], 0.0)
            blk_tiles = {}
            needed_blocks = sorted({s[0] for gs in segs for s in gs})
            for blk in needed_blocks:
                bt = wpool.tile([128, POOL_BLOCK], bf, tag="poolblk")
                nc.sync.dma_start(out=bt[:],
                                  in_=out4[blk * POOL_BLOCK:(blk + 1) * POOL_BLOCK, :],
                                  transpose=True)
                blk_tiles[blk] = bt
                # process all graphs fully contained up to this block now
                for g, gs in enumerate(segs):
                    if not gs or gs[-1][0] != blk:
                        continue
                    if len(gs) == 1:
                        b0, c0, wd = gs[0]
                        nc.vector.reduce_sum(sumsT[:, g:g + 1],
                                             blk_tiles[b0][:, c0:c0 + wd], axis=AX)
                        nc.vector.reduce_max(mxT[:, g:g + 1],
                                             blk_tiles[b0][:, c0:c0 + wd], axis=AX)
                    else:
                        tmp_s = wpool.tile([128, 1], f32, tag="tmp_s")
                        tmp_m = wpool.tile([128, 1], f32, tag="tmp_m")
                        first = True
                        for (b0, c0, wd) in gs:
                            if first:
                                nc.vector.reduce_sum(sumsT[:, g:g + 1],
                                                     blk_tiles[b0][:, c0:c0 + wd], axis=AX)
                                nc.vector.reduce_max(mxT[:, g:g + 1],
                                                     blk_tiles[b0][:, c0:c0 + wd], axis=AX)
                                first = False
                            else:
                                nc.vector.reduce_sum(tmp_s[:],
                                                     blk_tiles[b0][:, c0:c0 + wd], axis=AX)
                                nc.vector.reduce_max(tmp_m[:],
                                                     blk_tiles[b0][:, c0:c0 + wd], axis=AX)
                                nc.vector.tensor_add(sumsT[:, g:g + 1],
                                                     sumsT[:, g:g + 1], tmp_s[:])
                                nc.vector.tensor_max(mxT[:, g:g + 1],
                                                     mxT[:, g:g + 1], tmp_m[:])
            sumsTb = ppool.tile([128, N_GRAPHS], bf)
            mxTb = ppool.tile([128, N_GRAPHS], bf)
            nc.vector.tensor_copy(sumsTb[:], sumsT[:])
            nc.vector.tensor_copy(mxTb[:], mxT[:])

            for gt in range(N_GRAPHS // 128):
                p1 = pspool.tile([128, 6], f32, tag="nd_ps", space="PSUM")
                p2 = pspool.tile([128, 6], f32, tag="ad_e", space="PSUM")
                nc.tensor.matmul(p1[:], sumsTb[:, gt * 128:(gt + 1) * 128], fcWm[:],
                                 start=True, stop=True)
                nc.tensor.matmul(p2[:], mxTb[:, gt * 128:(gt + 1) * 128], fcWx[:],
                                 start=True, stop=True)
                zz = wpool.tile([128, 6], f32, tag="zz")
                nc.vector.tensor_mul(zz[:], p1[:],
                                     rcp_t[:, gt:gt + 1].to_broadcast([128, 6]))
                nc.vector.tensor_add(zz[:], zz[:], p2[:])
                nc.vector.tensor_add(zz[:], zz[:], fcb[:])
                m6 = wpool.tile([128, 1], f32, tag="m6")
                nc.vector.reduce_max(m6[:], zz[:], axis=AX)
                nc.vector.tensor_sub(zz[:], zz[:], m6[:].to_broadcast([128, 6]))
                e6 = wpool.tile([128, 6], f32, tag="e6")
                nc.scalar.activation(e6[:], zz[:], ACT.Exp)
                s6 = wpool.tile([128, 1], f32, tag="s6")
                nc.vector.reduce_sum(s6[:], e6[:], axis=AX)
                nc.scalar.activation(s6[:], s6[:], ACT.Ln)
                nc.vector.tensor_sub(zz[:], zz[:], s6[:].to_broadcast([128, 6]))
                nc.sync.dma_start(out=t_out[gt * 128:(gt + 1) * 128, :], in_=zz[:])
    nc.compile()
    return nc


def _kernel_trn(x, edge_index, batch,
                W1, a1s, a1d, b1, W2, a2s, a2d, b2,
                W3, a3s, a3d, b3, W4, a4s, a4d, b4, fcW, fcb):
    global _LAST_EXEC_NS
    from concourse.bass_utils import run_bass_kernel_spmd

    p = _prep(np.asarray(x), np.asarray(edge_index), np.asarray(batch),
              [W1, W2, W3, W4], [a1d, a2d, a3d, a4d], [a1s, a2s, a3s, a4s],
              [b1, b2, b3, b4], fcW, fcb)
    nc = _build_program(p)

    in_maps = []
    for cidx in range(NCORES):
        m = dict(
            x0T=p['x0T'][cidx], idx=p['idx_w'][cidx], drel=p['drel_w'][cidx],
            iota_f=p['consts']['iota_f'], iota_p=p['consts']['iota_p'],
            ident=p['consts']['ident'], fcWm=p['consts']['fcWm'],
            fcWx=p['consts']['fcWx'], fcb=p['consts']['fcb'],
            rcp=p['consts']['rcp_t'],
        )
        for li, L in enumerate(p['layers']):
            m[f"W{li}"] = L['W']
            m[f"WAd{li}"] = L['WAd']
            m[f"Asb{li}"] = L['As_b']
            m[f"bias{li}"] = L['bias']
        in_maps.append(m)

    trace = bool(int(os.environ.get("GAT_TRACE", "0")))
    tdir = os.environ.get("GAT_TRACE_DIR") or None
    res = run_bass_kernel_spmd(nc, in_maps, core_ids=list(range(NCORES)),
                               trace=trace, tmpdir=tdir)
    _LAST_EXEC_NS = res.exec_time_ns
    globals()['_LAST_RES'] = res
    return np.asarray(res.results[0]["out"], dtype=np.float32)


# ----------------------------------------------------------------------------
# numpy fallback (previous baseline)
# ----------------------------------------------------------------------------

def _leaky_relu(v, slope=0.2):
    return np.where(v >= 0, v, slope * v)


def _elu(v):
    return np.where(v > 0, v, np.expm1(np.minimum(v, 0.0)))


def _gat_layer_np(x, W, a_src, a_dst, b, src_s, dst_s, starts, n, c):
    h = (x @ W).reshape(n, H, c)
    al_s = np.einsum('nhc,hc->nh', h, a_src)
    al_d = np.einsum('nhc,hc->nh', h, a_dst)
    e = _leaky_relu(al_s[src_s] + al_d[dst_s])
    e_max = np.maximum.reduceat(e, starts, axis=0)
    ex = np.exp(e - e_max[dst_s])
    denom = np.add.reduceat(ex, starts, axis=0)
    alpha = ex / denom[dst_s]
    msg = h[src_s]
    msg *= alpha[:, :, None]
    out = np.add.reduceat(msg.reshape(-1, H * c), starts, axis=0)
    return out + b


def _kernel_numpy(x, edge_index, batch,
                  W1, a1s, a1d, b1, W2, a2s, a2d, b2,
                  W3, a3s, a3d, b3, W4, a4s, a4d, b4, fcW, fcb):
    x = np.asarray(x, dtype=np.float32)
    edge_index = np.asarray(edge_index)
    batch = np.asarray(batch)
    n = x.shape[0]
    loops = np.arange(n, dtype=edge_index.dtype)
    src = np.concatenate([edge_index[0], loops])
    dst = np.concatenate([edge_index[1], loops])
    order = np.argsort(dst, kind='stable')
    src_s = src[order]
    dst_s = dst[order]
    counts = np.bincount(dst, minlength=n)
    starts = np.zeros(n, dtype=np.int64)
    np.cumsum(counts[:-1], out=starts[1:])
    x = _elu(_gat_layer_np(x, W1, a1s, a1d, b1, src_s, dst_s, starts, n, 8))
    x = _elu(_gat_layer_np(x, W2, a2s, a2d, b2, src_s, dst_s, starts, n, 16))
    x = _elu(_gat_layer_np(x, W3, a3s, a3d, b3, src_s, dst_s, starts, n, 16))
    x = _elu(_gat_layer_np(x, W4, a4s, a4d, b4, src_s, dst_s, starts, n, 16))
    cnt = np.bincount(batch, minlength=N_GRAPHS)
    nz = cnt > 0
    bstarts = np.zeros(N_GRAPHS, dtype=np.int64)
    np.cumsum(cnt[:-1], out=bstarts[1:])
    f = x.shape[1]
    mean = np.zeros((N_GRAPHS, f), dtype=np.float32)
    mx = np.zeros((N_GRAPHS, f), dtype=np.float32)
    nz_starts = bstarts[nz]
    mean[nz] = np.add.reduceat(x, nz_starts, axis=0) / cnt[nz, None]
    mx[nz] = np.maximum.reduceat(x, nz_starts, axis=0)
    feat = np.concatenate([mean, mx], axis=1)
    z = feat @ fcW + fcb
    z -= z.max(axis=1, keepdims=True)
    z -= np.log(np.exp(z).sum(axis=1, keepdims=True))
    return z.astype(np.float32)


def kernel(**inputs):
    if not int(os.environ.get("GAT_FORCE_NUMPY", "0")):
        try:
            return _kernel_trn(**inputs)
        except Exception:
            import traceback
            traceback.print_exc()
    return _kernel_numpy(**inputs)

